# revision 1
# baseline (speedup 1.0000x reference)
"""Trainium2 Bass kernel for nn_BuddingLayer (moe_routing).

Computation (B=512, SIN=SOUT=2048, K=128 buds):
  dense = (x * ~mask) @ weight.T + bias          mask = one-hot(sat_idx)
  per bud k (v = x[:, sat_idx[k]]):
    h1 = relu(v * c1[k] + b1[k])                 c1[k,j] = sum_i W1[k,i,j]/3
    h2 = relu(h1 @ W2[k] + b2[k])                [B, 3]
    u += relu(h2 @ W3[k] + b3[k])                [B, 2048]
  out = dense + u

Sharding: output-feature split, 256 columns per core (8 cores), compute in
transposed layout [o_part, b_free].  Host does slicing/transposition only;
all math (masking, fp16 casts, c1 reduction) runs on device.

Bud path: one bud per 32-row PE group; super-tile t packs buds 4t..4t+3 at
row-group bases {0,32,64,96}.  K=4 matmul per (bud, o-chunk) with a
constant-1.0 4th rhs row whose lhsT row carries b3 (bias folded into the MM).
PSUM fp32 -> relu exits (ScalarE activation 3:1 VectorE tensor_scalar_max)
-> fp16 -> 16-bud block tree-sums (VectorE, a few blocks on GpSimd)
-> + dense (fp16 matmuls, x/w cast in-flight by SWDGE DMA) -> DMA out.
"""

import numpy as np

N_CORES = 8
B = 512
SIN = 2048
SOUT = 2048
K = 128
OC = SOUT // N_CORES          # 256 output cols per core
NCHUNK = SIN // 128           # 16 contraction chunks for dense
NT = K // 4                   # 32 super-tiles
BLK = 16                      # buds per tree block
NBLK = K // BLK               # 8 blocks per o-chunk

# tuning knobs
ACT_UNITS_OF_4 = 3            # of every 4 exit-units, this many go to ScalarE
GP_SUM_BLOCKS = 3             # tree blocks (of 16) summed on GpSimd (rest VectorE)

_compiled = {}


def _build(chunk_status, repeat=1):
    """Build the SPMD Bass program.  chunk_status: tuple of 'full'|'partial'|'clean'
    per 128-row input chunk ('full' = entirely masked, skip).  repeat>1 emits
    the whole body multiple times (benchmarking only)."""
    import concourse.bacc as bacc
    import concourse.mybir as mybir
    import concourse.tile as tile

    f32, f16 = mybir.dt.float32, mybir.dt.float16
    AL = mybir.AluOpType
    AF = mybir.ActivationFunctionType

    nc = bacc.Bacc("TRN2", target_bir_lowering=False, debug=False,
                   num_devices=N_CORES)

    # ---- DRAM I/O (per core) ----
    xT = nc.dram_tensor("xT", [SIN, B], f32, kind="ExternalInput")
    xsatT = nc.dram_tensor("xsatT", [K, B], f32, kind="ExternalInput")
    maskT = nc.dram_tensor("maskT", [SIN, 1], f32, kind="ExternalInput")
    wT = nc.dram_tensor("wT", [SIN, OC], f32, kind="ExternalInput")
    biasc = nc.dram_tensor("biasc", [1, OC], f32, kind="ExternalInput")
    w1d = nc.dram_tensor("w1d", [K, 9], f32, kind="ExternalInput")
    b1d = nc.dram_tensor("b1d", [K, 3], f32, kind="ExternalInput")
    w2d = nc.dram_tensor("w2d", [K, 9], f32, kind="ExternalInput")
    b2d = nc.dram_tensor("b2d", [K, 3], f32, kind="ExternalInput")
    w3d = nc.dram_tensor("w3d", [K, 3 * OC], f32, kind="ExternalInput")
    b3d = nc.dram_tensor("b3d", [K, OC], f32, kind="ExternalInput")
    outT = nc.dram_tensor("outT", [OC, B], f32, kind="ExternalOutput")

    with tile.TileContext(nc) as tc:
      for _rep in range(repeat):
        with (
            tc.tile_pool(name="const", bufs=1) as cp,
            tc.tile_pool(name="stage", bufs=3) as stp,
            tc.tile_pool(name="rblk", bufs=3) as rbp,
            tc.tile_pool(name="psum", bufs=3, space="PSUM") as pp,
            tc.tile_pool(name="psumd", bufs=1, space="PSUM") as ppd,
        ):
            # ---------- tiny constants ----------
            w1s = cp.tile([K, 9], f32)
            nc.sync.dma_start(w1s[:], w1d.ap())
            b1s = cp.tile([K, 3], f32)
            nc.sync.dma_start(b1s[:], b1d.ap())
            w2s = cp.tile([K, 9], f32)
            nc.sync.dma_start(w2s[:], w2d.ap())
            b2s = cp.tile([K, 3], f32)
            nc.sync.dma_start(b2s[:], b2d.ap())
            masks = cp.tile([128, NCHUNK], f32)
            nc.sync.dma_start(masks[:], maskT.ap().rearrange("(c p) one -> p (c one)", p=128))
            bias_sb = cp.tile([1, OC], f16)
            nc.gpsimd.dma_start(bias_sb[:], biasc.ap())

            # prefetch the ACT relu table set while input DMAs run
            warm = cp.tile([1, 1], f32)
            nc.scalar.activation(warm[:], w1s[0:1, 0:1], AF.Relu)

            # c1[k, j] = (W1[k,0,j] + W1[k,1,j] + W1[k,2,j]) / 3
            c1a = cp.tile([K, 3], f32)
            nc.vector.tensor_tensor(c1a[:], w1s[:, 0:3], w1s[:, 3:6], AL.add)
            c1 = cp.tile([K, 3], f32)
            nc.vector.tensor_tensor(c1[:], c1a[:], w1s[:, 6:9], AL.add)
            c1s = cp.tile([K, 3], f32)
            nc.vector.tensor_scalar_mul(c1s[:], c1[:], 1.0 / 3.0)

            # ---------- h path: v -> h1 -> h2 (layout [k, b]) ----------
            v = cp.tile([K, B], f32)
            nc.sync.dma_start(v[:], xsatT.ap())
            h1 = [cp.tile([K, B], f16, tag=f"h1_{j}", name=f"h1_{j}") for j in range(3)]
            for j in range(3):
                nc.scalar.activation(h1[j][:], v[:], AF.Relu,
                                     bias=b1s[:, j:j + 1], scale=c1s[:, j:j + 1])
            h2 = [cp.tile([K, B], f16, tag=f"h2_{j}", name=f"h2_{j}") for j in range(3)]
            for j in range(3):
                ma = stp.tile([K, B], f16, tag="hm0", name=f"hma{j}")
                nc.vector.tensor_scalar_mul(ma[:], h1[0][:], w2s[:, j : j + 1])
                mb = stp.tile([K, B], f16, tag="hm1", name=f"hmb{j}")
                nc.vector.tensor_scalar_mul(mb[:], h1[1][:], w2s[:, 3 + j : 4 + j])
                sab = stp.tile([K, B], f16, tag="hm0", name=f"hsab{j}")
                nc.vector.tensor_tensor(sab[:], ma[:], mb[:], AL.add)
                mc = stp.tile([K, B], f16, tag="hm1", name=f"hmc{j}")
                nc.vector.tensor_scalar_mul(mc[:], h1[2][:], w2s[:, 6 + j : 7 + j])
                s = stp.tile([K, B], f16, tag="hm0", name=f"hs{j}")
                nc.vector.tensor_tensor(s[:], sab[:], mc[:], AL.add)
                nc.scalar.activation(h2[j][:], s[:], AF.Relu, bias=b2s[:, j:j + 1])

            # ---------- W3B / b3 -> packed fp16, H2T packed fp16 ----------
            w3s = cp.tile([K, 3 * OC], f16)
            nc.gpsimd.dma_start(w3s[:], w3d.ap())
            b3s = cp.tile([K, OC], f16)
            nc.gpsimd.dma_start(b3s[:], b3d.ap())
            ones32 = cp.tile([32, 512], f16)
            nc.vector.memset(ones32[:], 1.0)

            h2t = cp.tile([128, 512 * NT], f16)       # [128, 16384]
            w3b = cp.tile([128, OC * NT], f16)        # [128, 8192]
            for g in range(4):
                # split rearranges across HWDGE and SWDGE so descriptor
                # processing runs in parallel
                eng_a = nc.sync if g % 2 == 0 else nc.gpsimd
                eng_b = nc.gpsimd if g % 2 == 0 else nc.sync
                for j in range(3):
                    eng_a.dma_start(
                        h2t[32 * g + j : 32 * g + j + 1, :].rearrange("p (t b) -> p t b", b=512),
                        h2[j][g::4, :],
                    )
                    eng_b.dma_start(
                        w3b[32 * g + j : 32 * g + j + 1, :].rearrange("p (t o) -> p t o", o=OC),
                        w3s[g::4, OC * j : OC * (j + 1)],
                    )
                eng_a.dma_start(
                    h2t[32 * g + 3 : 32 * g + 4, :].rearrange("p (t b) -> p t b", b=512),
                    ones32[:],
                )
                eng_b.dma_start(
                    w3b[32 * g + 3 : 32 * g + 4, :].rearrange("p (t o) -> p t o", o=OC),
                    b3s[g::4, :],
                )

            # ---------- dense inputs: one cast-DMA each for x and w ----------
            live = [c for c in range(NCHUNK) if chunk_status[c] != "full"]
            x16a = cp.tile([128, 512 * NCHUNK], f16)
            w16a = cp.tile([128, OC * NCHUNK], f16)
            input_dmas = [False]

            def emit_input_dmas():
                # deferred so the startup SWDGE queue serves the bud path first
                if input_dmas[0]:
                    return
                input_dmas[0] = True
                nc.gpsimd.dma_start(
                    x16a[:].rearrange("p (c b) -> p c b", b=B),
                    xT.ap().rearrange("(c p) b -> p c b", p=128))
                nc.gpsimd.dma_start(
                    w16a[:].rearrange("p (c o) -> p c o", o=OC),
                    wT.ap().rearrange("(c p) o -> p c o", p=128))
            dps = ppd.tile([128, 1024], f32, name="dps")  # [:, :512]=oc0, [:, 512:]=oc1
            dense_state = {"first": True}

            def emit_dense_chunk(c):
                x16 = x16a[:, 512 * c : 512 * (c + 1)]
                if chunk_status[c] == "partial":
                    xm = stp.tile([128, B], f16, tag="x16m", name=f"x16m_{c}_{_rep}")
                    nc.vector.tensor_scalar_mul(xm[:], x16, masks[:, c : c + 1])
                    x16 = xm[:]
                for oc in range(2):
                    nc.tensor.matmul(dps[:, 512 * oc : 512 * (oc + 1)],
                                     w16a[:, OC * c + 128 * oc : OC * c + 128 * oc + 128],
                                     x16,
                                     start=dense_state["first"], stop=False)
                dense_state["first"] = False

            pending = list(live)

            # ---------- bud matmuls + relu exits + block trees ----------
            blocksums = cp.tile([128, 512 * NBLK * 2], f16)   # [oc, blk]
            vscr = cp.tile([128, 4096 + 2048 + 1024 + 2048], f16)
            fscr = cp.tile([128, 2048 + 1024 + 1024], f16)
            gscr = cp.tile([128, 4096 + 2048 + 1024], f16)
            outsb = cp.tile([128, 1024], f32)
            unit_counter = [0]
            n_blocks = 2 * NBLK
            modes = ["dve"] * n_blocks
            for i in range(GP_SUM_BLOCKS):
                modes[(i * n_blocks) // max(GP_SUM_BLOCKS, 1) % n_blocks] = "gp"
            blk_idx = 0
            for oc in range(2):
                for blk in range(NBLK):
                    rb = rbp.tile([128, 512 * BLK], f16, tag="rblk", name=f"rb{oc}_{blk}")
                    for u in range(BLK // 2):         # 2-bud exit units
                        k0 = BLK * blk + 2 * u
                        t = k0 // 4
                        g0 = k0 % 4                   # buds k0, k0+1 -> groups g0, g0+1
                        zps = pp.tile([128, 1024], f32, tag="zps", name=f"z{oc}_{blk}_{u}")
                        for d in range(2):
                            g = g0 + d
                            nc.tensor.matmul(
                                zps[:, 512 * d : 512 * (d + 1)],
                                w3b[32 * g : 32 * g + 4, OC * t + 128 * oc : OC * t + 128 * oc + 128],
                                h2t[32 * g : 32 * g + 4, 512 * t : 512 * (t + 1)],
                                start=True, stop=True, tile_position=(32 * g, 0),
                            )
                        dst = rb[:, 1024 * u : 1024 * (u + 1)]
                        if (u % 4) < ACT_UNITS_OF_4:
                            nc.scalar.activation(dst, zps[:], AF.Relu)
                        else:
                            nc.vector.tensor_scalar_max(dst, zps[:], 0.0)
                        unit_counter[0] += 1
                        if unit_counter[0] == 8:
                            emit_input_dmas()
                        if unit_counter[0] % 8 == 0 and pending:
                            emit_dense_chunk(pending.pop(0))
                    # tree-sum the 16 buds of this block -> blocksums col
                    bs = blocksums[:, 512 * blk_idx : 512 * (blk_idx + 1)]
                    mode = modes[blk_idx]
                    eng = nc.gpsimd if mode == "gp" else nc.vector
                    scr = gscr if mode == "gp" else vscr
                    src, width, off = rb[:], 512 * BLK, 0
                    while width > 1024:
                        half = width // 2
                        dst_ = scr[:, off : off + half]
                        eng.tensor_tensor(dst_, src[:, 0:half], src[:, half:width], AL.add)
                        src, width, off = scr[:, off : off + half], half, off + half
                    eng.tensor_tensor(bs, src[:, 0:512], src[:, 512:1024], AL.add)
                    blk_idx += 1
                    if blk == NBLK - 1:
                        base = 512 * NBLK * oc
                        cur, width, off = blocksums[:, base : base + 512 * NBLK], 512 * NBLK, 0
                        while width > 1024:
                            half = width // 2
                            nc.vector.tensor_tensor(fscr[:, off : off + half], cur[:, 0:half],
                                                    cur[:, half:width], AL.add)
                            cur, width, off = fscr[:, off : off + half], half, off + half
                        nc.vector.tensor_tensor(fscr[:, 3072 + 512 * oc : 3072 + 512 * (oc + 1)],
                                                cur[:, 0:512], cur[:, 512:1024], AL.add)

            emit_input_dmas()
            for c in pending:
                emit_dense_chunk(c)
            for oc in range(2):   # bias row
                nc.tensor.matmul(dps[:, 512 * oc : 512 * (oc + 1)],
                                 bias_sb[:, 128 * oc : 128 * (oc + 1)],
                                 ones32[0:1, :], start=False, stop=True)
            dense_sb = cp.tile([128, 1024], f32)
            nc.vector.tensor_copy(dense_sb[:], dps[:])

            # ---------- final: add dense to early-computed bud roots, store ----------
            for oc in range(2):
                ft = fscr[:, 3072 + 512 * oc : 3072 + 512 * (oc + 1)]
                ot = outsb[:, 512 * oc : 512 * (oc + 1)]
                nc.vector.tensor_tensor(ot, dense_sb[:, 512 * oc : 512 * (oc + 1)], ft, AL.add)
                nc.sync.dma_start(outT.ap()[128 * oc : 128 * (oc + 1), :], ot)
    nc.finalize()
    return nc


def _prep_inputs(x, sat_idx, weight, bias, W1, b1, W2, b2, W3, b3):
    """Host-side shard/layout prep. Returns (chunk_status, per-core input maps)."""
    x = np.ascontiguousarray(np.asarray(x, np.float32))
    sat = np.asarray(sat_idx).astype(np.int64)
    weight = np.asarray(weight, np.float32)
    bias = np.asarray(bias, np.float32)

    mask = np.ones(SIN, np.float32)
    mask[sat] = 0.0
    chunk_status = []
    for c in range(NCHUNK):
        mc = mask[128 * c : 128 * (c + 1)]
        if not mc.any():
            chunk_status.append("full")
        elif mc.all():
            chunk_status.append("clean")
        else:
            chunk_status.append("partial")
    chunk_status = tuple(chunk_status)

    xT = np.ascontiguousarray(x.T)                       # [SIN, B]
    xsatT = np.ascontiguousarray(x[:, sat].T)            # [K, B]
    maskT = np.ascontiguousarray(mask[:, None])          # [SIN, 1]
    w1h = np.ascontiguousarray(np.asarray(W1, np.float32).reshape(K, 9))
    w2h = np.ascontiguousarray(np.asarray(W2, np.float32).reshape(K, 9))
    b1h = np.ascontiguousarray(np.asarray(b1, np.float32))
    b2h = np.ascontiguousarray(np.asarray(b2, np.float32))
    W3 = np.asarray(W3, np.float32)
    b3 = np.asarray(b3, np.float32)

    in_maps = []
    for c in range(N_CORES):
        sl = slice(OC * c, OC * (c + 1))
        in_maps.append({
            "xT": xT,
            "xsatT": xsatT,
            "maskT": maskT,
            "wT": np.ascontiguousarray(weight[sl, :].T),          # [SIN, OC]
            "biasc": np.ascontiguousarray(bias[sl][None, :]),     # [1, OC]
            "w1d": w1h, "b1d": b1h, "w2d": w2h, "b2d": b2h,
            "w3d": np.ascontiguousarray(W3[:, :, sl].reshape(K, 3 * OC)),
            "b3d": np.ascontiguousarray(b3[:, sl]),
        })
    return chunk_status, in_maps


def kernel(**inputs) -> np.ndarray:
    from concourse.bass_utils import run_bass_kernel_spmd

    chunk_status, in_maps = _prep_inputs(
        inputs["x"], inputs["sat_idx"], inputs["weight"], inputs["bias"],
        inputs["W1"], inputs["b1"], inputs["W2"], inputs["b2"],
        inputs["W3"], inputs["b3"],
    )
    if chunk_status not in _compiled:
        _compiled[chunk_status] = _build(chunk_status)
    nc = _compiled[chunk_status]
    res = run_bass_kernel_spmd(nc, in_maps, core_ids=list(range(N_CORES)))
    outT = np.concatenate([res.results[c]["outT"] for c in range(N_CORES)], axis=0)
    return np.ascontiguousarray(outT.T).astype(np.float32)



# revision 42
# speedup vs baseline: 1.1332x; 1.1332x over previous
"""Trainium2 Bass kernel for nn_BuddingLayer (moe_routing).

Computation (B=512, SIN=SOUT=2048, K=128 buds):
  dense = (x * ~mask) @ weight.T + bias          mask = one-hot(sat_idx)
  per bud k (v = x[:, sat_idx[k]]):
    h1 = relu(v * c1[k] + b1[k])                 c1[k,j] = sum_i W1[k,i,j]/3
    h2 = relu(h1 @ W2[k] + b2[k])                [B, 3]
    u += relu(h2 @ W3[k] + b3[k])                [B, 2048]
  out = dense + u

Sharding: output-feature split, 256 columns per core (8 cores); compute in
transposed layout [o_part, b_free].  Host does slicing/permutation only; all
math runs on device.

v2 layout: supertile t packs buds {t, 32+t, 64+t, 96+t} at PE row-group bases
{0,32,64,96}; 4-row lhsT per bud carries W3 rows + b3 (constant-1 rhs row).
Dense runs first in its own PSUM scope; after it drains, bud z-units are
[128, 2048] (4 buds x 4 PSUM banks, double buffered).  Exits split between
ScalarE (relu -> fp16 slab slots, summed by SWDGE cce-add DMA trees + chained
DMA accumulate) and VectorE (fused relu+accumulate via scalar_tensor_tensor
into fp32 lanes).  Lane folds and final dense+bud combines run on GpSimd.
"""

import numpy as np

N_CORES = 8
B = 512
SIN = 2048
SOUT = 2048
K = 128
OC = SOUT // N_CORES          # 256 output cols per core
NCHUNK = SIN // 128           # 16 contraction chunks for dense
NT = K // 4                   # 32 supertiles (4 buds each)
NU = 2 * NT                   # 64 z-units: unit u -> oc=u%2, t=u//2

# tuning knobs
ACT_EXITS = 68                # [128,1024] exits via ScalarE (rest: DVE fused)
TAILU = 4                     # per-oc trailing units grouped oc0-then-oc1
TAIL_D_UNITS = 2              # last units per oc forced to DVE (no DMA tail)
GRP = 4                       # Act slots (1024 wide) per cce-add chain DMA
RING = 12                     # slab-slot ring depth per oc

_compiled = {}


def _unit_order():
    return ([(t, oc) for t in range(NT - TAILU) for oc in (0, 1)]
            + [(t, 0) for t in range(NT - TAILU, NT)]
            + [(t, 1) for t in range(NT - TAILU, NT)])


def _exit_modes():
    """Per-exit 'A' (ScalarE relu -> slab slot) / 'D' (DVE fused relu+acc).
    Bresenham interleave keeps runs short so neither engine bursts; the last
    units of each oc go to DVE so no DMA chain dangles at the end."""
    order = _unit_order()
    ne = 2 * NU
    tail_units = set()
    for oc in (0, 1):
        idxs = [i for i, (t, o) in enumerate(order) if o == oc]
        tail_units.update(idxs[-TAIL_D_UNITS:])
    modes = ["D"] * ne
    body = [e for e in range(ne) if (e // 2) not in tail_units]
    n_act = min(ACT_EXITS, len(body))
    acc = 0
    for e in body:
        acc += n_act
        if acc >= len(body):
            acc -= len(body)
            modes[e] = "A"
    return modes


def _build(chunk_status, repeat=1):
    import concourse.bacc as bacc
    import concourse.mybir as mybir
    import concourse.tile as tile

    f32, f16 = mybir.dt.float32, mybir.dt.float16
    AL = mybir.AluOpType
    AF = mybir.ActivationFunctionType

    nc = bacc.Bacc("TRN2", target_bir_lowering=False, debug=False,
                   num_devices=N_CORES)

    # ---- DRAM I/O (per core) ----
    xT = nc.dram_tensor("xT", [SIN, B], f32, kind="ExternalInput")
    xsatT = nc.dram_tensor("xsatT", [K, B], f32, kind="ExternalInput")
    maskT = nc.dram_tensor("maskT", [SIN, 1], f32, kind="ExternalInput")
    wT = nc.dram_tensor("wT", [SIN, OC], f32, kind="ExternalInput")
    bias2d = nc.dram_tensor("bias2d", [128, 2], f32, kind="ExternalInput")
    w1d = nc.dram_tensor("w1d", [K, 9], f32, kind="ExternalInput")
    b1d = nc.dram_tensor("b1d", [K, 3], f32, kind="ExternalInput")
    w2d = nc.dram_tensor("w2d", [K, 9], f32, kind="ExternalInput")
    b2d = nc.dram_tensor("b2d", [K, 3], f32, kind="ExternalInput")
    w3bd = nc.dram_tensor("w3bd", [128, NT * OC], f32, kind="ExternalInput")
    outT = nc.dram_tensor("outT", [OC, B], f32, kind="ExternalOutput")

    modes = _exit_modes()

    with tile.TileContext(nc) as tc:
      for _rep in range(repeat):
        with (
            tc.tile_pool(name="const", bufs=1) as cp,
            tc.tile_pool(name="stage", bufs=3) as stp,
            tc.tile_pool(name="psumz", bufs=4, space="PSUM") as pp,
        ):
            # ---------- input loads ----------
            # HWDGE: small fp32 tensors
            v = cp.tile([K, B], f32)
            nc.sync.dma_start(v[:], xsatT.ap())
            w1s = cp.tile([K, 9], f32)
            nc.sync.dma_start(w1s[:], w1d.ap())
            b1s = cp.tile([K, 3], f32)
            nc.sync.dma_start(b1s[:], b1d.ap())
            w2s = cp.tile([K, 9], f32)
            nc.sync.dma_start(w2s[:], w2d.ap())
            b2s = cp.tile([K, 3], f32)
            nc.sync.dma_start(b2s[:], b2d.ap())
            bias2 = cp.tile([128, 2], f32)
            nc.sync.dma_start(bias2[:], bias2d.ap())
            masks = cp.tile([128, NCHUNK], f32)
            nc.sync.dma_start(masks[:], maskT.ap().rearrange("(c p) one -> p (c one)", p=128))

            # SWDGE cast loads (fp32 DRAM -> fp16 SBUF)
            x16a = cp.tile([128, 512 * NCHUNK], f16)
            w16a = cp.tile([128, OC * NCHUNK], f16)
            w3b = cp.tile([128, NT * OC], f16)
            QC = NCHUNK // 4

            def load_xw_quarter(q):
                nc.gpsimd.dma_start(
                    x16a[:, 512 * QC * q : 512 * QC * (q + 1)]
                        .rearrange("p (c b) -> p c b", b=B),
                    xT.ap()[128 * QC * q : 128 * QC * (q + 1), :]
                        .rearrange("(c p) b -> p c b", p=128))
                nc.gpsimd.dma_start(
                    w16a[:, OC * QC * q : OC * QC * (q + 1)]
                        .rearrange("p (c o) -> p c o", o=OC),
                    wT.ap()[128 * QC * q : 128 * QC * (q + 1), :]
                        .rearrange("(c p) o -> p c o", p=128))

            load_xw_quarter(0)
            load_xw_quarter(1)
            nc.gpsimd.dma_start(w3b[:], w3bd.ap())
            load_xw_quarter(2)
            load_xw_quarter(3)

            # prefetch the ACT relu table while DMAs run
            warm = cp.tile([1, 1], f32)
            nc.scalar.activation(warm[:], w1s[0:1, 0:1], AF.Relu)

            # c1[k, j] = (W1[k,0,j] + W1[k,1,j] + W1[k,2,j]) / 3
            c1a = cp.tile([K, 3], f32)
            nc.vector.tensor_tensor(c1a[:], w1s[:, 0:3], w1s[:, 3:6], AL.add)
            c1 = cp.tile([K, 3], f32)
            nc.vector.tensor_tensor(c1[:], c1a[:], w1s[:, 6:9], AL.add)
            c1s = cp.tile([K, 3], f32)
            nc.vector.tensor_scalar_mul(c1s[:], c1[:], 1.0 / 3.0)

            # ---------- h path: v -> h1 -> h2 (layout [k, b]) ----------
            h1 = [cp.tile([K, B], f16, tag=f"h1_{j}", name=f"h1_{j}") for j in range(3)]
            for j in range(3):
                nc.scalar.activation(h1[j][:], v[:], AF.Relu,
                                     bias=b1s[:, j:j + 1], scale=c1s[:, j:j + 1])
            h2 = [cp.tile([K, B], f16, tag=f"h2_{j}", name=f"h2_{j}") for j in range(3)]
            for j in range(3):
                ma = stp.tile([K, B], f16, tag="hm0", name=f"hma{j}")
                nc.vector.tensor_scalar_mul(ma[:], h1[0][:], w2s[:, j : j + 1])
                mb = stp.tile([K, B], f16, tag="hm1", name=f"hmb{j}")
                nc.vector.tensor_scalar(mb[:], h1[1][:], w2s[:, 3 + j : 4 + j],
                                        b2s[:, j : j + 1], AL.mult, AL.add)
                sab = stp.tile([K, B], f16, tag="hm2", name=f"hsab{j}")
                nc.vector.tensor_tensor(sab[:], ma[:], mb[:], AL.add)
                mc = stp.tile([K, B], f16, tag="hm1", name=f"hmc{j}")
                nc.vector.tensor_scalar_mul(mc[:], h1[2][:], w2s[:, 6 + j : 7 + j])
                s = stp.tile([K, B], f16, tag="hm0", name=f"hs{j}")
                nc.vector.tensor_tensor(s[:], sab[:], mc[:], AL.add)
                nc.vector.tensor_scalar_max(h2[j][:], s[:], 0.0)

            # ---------- h2t pack: row 32g+j <- h2[j] (buds 32g+t), 4 DMAs ----
            h2t = cp.tile([128, 512 * NT], f16)       # [128, 16384]
            ones128 = cp.tile([128, 512], f16)
            nc.vector.memset(ones128[:], 1.0)
            for j in range(3):
                nc.sync.dma_start(
                    h2t[j::32, :].rearrange("p (t b) -> p t b", b=512),
                    h2[j][:],
                )
            nc.sync.dma_start(
                h2t[3::32, :].rearrange("p (t b) -> p t b", b=512),
                ones128[:],
            )

            # ---------- dense: one ring tile, chunk groups interleaved -------
            live = [c for c in range(NCHUNK) if chunk_status[c] != "full"]
            dsb = cp.tile([128, 1024], f32)
            dps = pp.tile([128, 1024], f32, tag="z", name="dense")
            dgroups = [[c for c in live if c < 8],
                       [c for c in live if 8 <= c < 12],
                       [c for c in live if c >= 12]]

            def emit_dense(gi):
                for c in dgroups[gi]:
                    x16 = x16a[:, 512 * c : 512 * (c + 1)]
                    if chunk_status[c] == "partial":
                        xm = stp.tile([128, B], f16, tag="x16m", name=f"x16m_{c}_{_rep}")
                        nc.vector.tensor_scalar_mul(xm[:], x16, masks[:, c : c + 1])
                        x16 = xm[:]
                    for oc in range(2):
                        nc.tensor.matmul(dps[:, 512 * oc : 512 * (oc + 1)],
                                         w16a[:, OC * c + 128 * oc : OC * c + 128 * oc + 128],
                                         x16,
                                         start=c == live[0], stop=c == live[-1])

            def emit_dsb():
                for oc in range(2):
                    nc.scalar.activation(dsb[:, 512 * oc : 512 * (oc + 1)],
                                         dps[:, 512 * oc : 512 * (oc + 1)],
                                         AF.Identity, bias=bias2[:, oc : oc + 1])

            # ---------- bud units ----------
            acc32 = cp.tile([128, 4096], f32)         # 2 oc x 2 lanes x 1024
            acc16 = cp.tile([128, 4096], f16)         # 2 oc x 2 lanes x 1024
            rbslab = cp.tile([128, 2 * RING * 1024], f16)   # 2 oc x RING slots
            n_act = [0, 0]
            first_d = [True, True, True, True]
            first_chain = [True, True]
            odd_slots = {0: [], 1: []}

            def emit_chain(oc, base, nslots):
                # 2-slot cce-add sub-DMAs folding slab slots into acc16 lanes
                for s in range(0, nslots - 1, 2):
                    off = (oc * RING + base + s) * 1024
                    rb = rbslab[:, off : off + 2048]
                    aoc = acc16[:, 2048 * oc : 2048 * (oc + 1)]
                    if first_chain[oc]:
                        nc.gpsimd.dma_start(aoc, rb)
                        first_chain[oc] = False
                    else:
                        nc.gpsimd.dma_start(aoc, rb, accum_op=AL.add)
                if nslots % 2:
                    odd_slots[oc].append(base + nslots - 1)

            outsb = cp.tile([128, 1024], f32)

            def emit_tail(oc):
                # leftover chain, lane folds + final combine on DVE, store.
                nleft = n_act[oc] % GRP
                if nleft:
                    emit_chain(oc, (n_act[oc] - nleft) % RING, nleft)
                a16 = acc16[:, 2048 * oc : 2048 * (oc + 1)]
                for slot in odd_slots[oc]:
                    off = (oc * RING + slot) * 1024
                    nc.vector.tensor_tensor(a16[:, 0:1024], a16[:, 0:1024],
                                            rbslab[:, off : off + 1024], AL.add)
                nc.vector.tensor_tensor(a16[:, 0:1024], a16[:, 0:1024],
                                        a16[:, 1024:2048], AL.add)
                nc.vector.tensor_tensor(a16[:, 0:512], a16[:, 0:512],
                                        a16[:, 512:1024], AL.add)
                a32 = acc32[:, 2048 * oc : 2048 * (oc + 1)]
                nc.vector.tensor_tensor(a32[:, 0:1024], a32[:, 0:1024],
                                        a32[:, 1024:2048], AL.add)
                nc.vector.tensor_tensor(a32[:, 0:512], a32[:, 0:512],
                                        a32[:, 512:1024], AL.add)
                nc.vector.tensor_tensor(a32[:, 0:512], a32[:, 0:512],
                                        a16[:, 0:512], AL.add)
                ot = outsb[:, 512 * oc : 512 * (oc + 1)]
                nc.vector.tensor_tensor(ot, dsb[:, 512 * oc : 512 * (oc + 1)],
                                        a32[:, 0:512], AL.add)
                nc.sync.dma_start(outT.ap()[128 * oc : 128 * (oc + 1), :], ot)

            unit_order = _unit_order()
            last_unit = {0: max(i for i, (t, o) in enumerate(unit_order) if o == 0),
                         1: len(unit_order) - 1}

            emit_dense(0)
            for i, (t, oc) in enumerate(unit_order):
                if i == 4:
                    emit_dense(1)
                if i == 10:
                    emit_dense(2)
                    emit_dsb()
                for half in range(2):
                    e = 2 * i + half
                    zt = pp.tile([128, 1024], f32, tag="z", name=f"z{i}_{half}")
                    for gi in range(2):
                        g = 2 * half + gi
                        nc.tensor.matmul(
                            zt[:, 512 * gi : 512 * (gi + 1)],
                            w3b[32 * g : 32 * g + 4,
                                OC * t + 128 * oc : OC * t + 128 * oc + 128],
                            h2t[32 * g : 32 * g + 4, 512 * t : 512 * (t + 1)],
                            start=True, stop=True, tile_position=(32 * g, 0),
                        )
                    if modes[e] == "A":
                        slot = n_act[oc] % RING
                        dst = rbslab[:, (oc * RING + slot) * 1024 :
                                        (oc * RING + slot + 1) * 1024]
                        nc.scalar.activation(dst, zt[:], AF.Relu)
                        n_act[oc] += 1
                        if n_act[oc] % GRP == 0:
                            emit_chain(oc, (n_act[oc] - GRP) % RING, GRP)
                    else:
                        lane = acc32[:, 2048 * oc + 1024 * half :
                                        2048 * oc + 1024 * (half + 1)]
                        if first_d[oc * 2 + half]:
                            nc.vector.tensor_scalar_max(lane, zt[:], 0.0)
                            first_d[oc * 2 + half] = False
                        else:
                            nc.vector.scalar_tensor_tensor(
                                lane, zt[:], 0.0, lane, op0=AL.max, op1=AL.add)
                if i == len(unit_order) - 3:
                    emit_tail(0)
                elif i == last_unit[1]:
                    emit_tail(1)
    nc.finalize()
    return nc


def _prep_inputs(x, sat_idx, weight, bias, W1, b1, W2, b2, W3, b3):
    """Host-side shard/layout prep (slicing/permutation only)."""
    x = np.ascontiguousarray(np.asarray(x, np.float32))
    sat = np.asarray(sat_idx).astype(np.int64)
    weight = np.asarray(weight, np.float32)
    bias = np.asarray(bias, np.float32)

    mask = np.ones(SIN, np.float32)
    mask[sat] = 0.0
    chunk_status = []
    for c in range(NCHUNK):
        mc = mask[128 * c : 128 * (c + 1)]
        if not mc.any():
            chunk_status.append("full")
        elif mc.all():
            chunk_status.append("clean")
        else:
            chunk_status.append("partial")
    chunk_status = tuple(chunk_status)

    xT = np.ascontiguousarray(x.T)                       # [SIN, B]
    xsatT = np.ascontiguousarray(x[:, sat].T)            # [K, B]
    maskT = np.ascontiguousarray(mask[:, None])          # [SIN, 1]
    w1h = np.ascontiguousarray(np.asarray(W1, np.float32).reshape(K, 9))
    w2h = np.ascontiguousarray(np.asarray(W2, np.float32).reshape(K, 9))
    b1h = np.ascontiguousarray(np.asarray(b1, np.float32))
    b2h = np.ascontiguousarray(np.asarray(b2, np.float32))
    W3 = np.asarray(W3, np.float32)
    b3 = np.asarray(b3, np.float32)

    in_maps = []
    for cidx in range(N_CORES):
        sl = slice(OC * cidx, OC * (cidx + 1))
        # w3b rows 32g+j hold W3[32g+t, j, sl] over supertiles t; row 32g+3 = b3
        w3bh = np.empty((128, NT, OC), np.float32)
        W3p = W3[:, :, sl]                                # [K, 3, OC]
        b3p = b3[:, sl]                                   # [K, OC]
        for g in range(4):
            buds = np.arange(NT) + 32 * g                 # k = 32g + t
            for j in range(3):
                w3bh[32 * g + j] = W3p[buds, j, :]
            w3bh[32 * g + 3] = b3p[buds, :]
        bias2 = np.ascontiguousarray(bias[sl].reshape(2, 128).T)  # [128, 2]
        in_maps.append({
            "xT": xT,
            "xsatT": xsatT,
            "maskT": maskT,
            "wT": np.ascontiguousarray(weight[sl, :].T),          # [SIN, OC]
            "bias2d": bias2,
            "w1d": w1h, "b1d": b1h, "w2d": w2h, "b2d": b2h,
            "w3bd": np.ascontiguousarray(w3bh.reshape(128, NT * OC)),
        })
    return chunk_status, in_maps


def kernel(**inputs) -> np.ndarray:
    from concourse.bass_utils import run_bass_kernel_spmd

    chunk_status, in_maps = _prep_inputs(
        inputs["x"], inputs["sat_idx"], inputs["weight"], inputs["bias"],
        inputs["W1"], inputs["b1"], inputs["W2"], inputs["b2"],
        inputs["W3"], inputs["b3"],
    )
    if chunk_status not in _compiled:
        _compiled[chunk_status] = _build(chunk_status)
    nc = _compiled[chunk_status]
    res = run_bass_kernel_spmd(nc, in_maps, core_ids=list(range(N_CORES)))
    outT = np.concatenate([res.results[c]["outT"] for c in range(N_CORES)], axis=0)
    return np.ascontiguousarray(outT.T).astype(np.float32)


# revision 54
# speedup vs baseline: 1.2442x; 1.0980x over previous
"""Trainium2 Bass kernel for nn_BuddingLayer (moe_routing).

Computation (B=512, SIN=SOUT=2048, K=128 buds):
  dense = (x * ~mask) @ weight.T + bias          mask = one-hot(sat_idx)
  per bud k (v = x[:, sat_idx[k]]):
    h1 = relu(v * c1[k] + b1[k])                 c1[k,j] = sum_i W1[k,i,j]/3
    h2 = relu(h1 @ W2[k] + b2[k])                [B, 3]
    u += relu(h2 @ W3[k] + b3[k])                [B, 2048]
  out = dense + u

Sharding: output-feature split, 256 columns per core (8 cores); compute in
transposed layout [o_part, b_free].  Host does slicing/permutation only; all
math runs on device.

v2 layout: supertile t packs buds {t, 32+t, 64+t, 96+t} at PE row-group bases
{0,32,64,96}; 4-row lhsT per bud carries W3 rows + b3 (constant-1 rhs row).
Dense runs first in its own PSUM scope; after it drains, bud z-units are
[128, 2048] (4 buds x 4 PSUM banks, double buffered).  Exits split between
ScalarE (relu -> fp16 slab slots, summed by SWDGE cce-add DMA trees + chained
DMA accumulate) and VectorE (fused relu+accumulate via scalar_tensor_tensor
into fp32 lanes).  Lane folds and final dense+bud combines run on GpSimd.
"""

import numpy as np

N_CORES = 8
B = 512
SIN = 2048
SOUT = 2048
K = 128
OC = SOUT // N_CORES          # 256 output cols per core
NCHUNK = SIN // 128           # 16 contraction chunks for dense
NT = K // 4                   # 32 supertiles (4 buds each)
NU = 2 * NT                   # 64 z-units: unit u -> oc=u%2, t=u//2

# tuning knobs
ACT_EXITS = 62                # body [128,1024] exits via ScalarE (rest: DVE)
TAILU = 4                     # per-oc trailing units grouped oc0-then-oc1
TAIL_A_UNITS = 2              # last units per oc -> Act relu + DVE TT folds
GRP = 4                       # Act slots (1024 wide) per cce-add chain DMA
RING = 12                     # slab-slot ring depth per oc

_compiled = {}


def _unit_order():
    return ([(t, oc) for t in range(NT - TAILU) for oc in (0, 1)]
            + [(t, 0) for t in range(NT - TAILU, NT)]
            + [(t, 1) for t in range(NT - TAILU, NT)])


def _exit_modes():
    """Per-exit 'A' (ScalarE relu -> slab slot) / 'D' (DVE fused relu+acc).
    Bresenham interleave keeps runs short so neither engine bursts; the last
    units of each oc go to DVE so no DMA chain dangles at the end."""
    order = _unit_order()
    ne = 2 * NU
    tail_units = set()
    for oc in (0, 1):
        idxs = [i for i, (t, o) in enumerate(order) if o == oc]
        tail_units.update(idxs[-TAIL_A_UNITS:])
    modes = ["D"] * ne
    body = [e for e in range(ne) if (e // 2) not in tail_units]
    n_act = min(ACT_EXITS, len(body))
    acc = 0
    for e in body:
        acc += n_act
        if acc >= len(body):
            acc -= len(body)
            modes[e] = "A"
    for u in tail_units:
        modes[2 * u] = modes[2 * u + 1] = "A"
    return modes


def _build(chunk_status, repeat=1):
    import concourse.bacc as bacc
    import concourse.mybir as mybir
    import concourse.tile as tile

    f32, f16 = mybir.dt.float32, mybir.dt.float16
    AL = mybir.AluOpType
    AF = mybir.ActivationFunctionType

    nc = bacc.Bacc("TRN2", target_bir_lowering=False, debug=False,
                   num_devices=N_CORES)

    # ---- DRAM I/O (per core) ----
    xT = nc.dram_tensor("xT", [SIN, B], f32, kind="ExternalInput")
    xsatT = nc.dram_tensor("xsatT", [K, B], f32, kind="ExternalInput")
    maskT = nc.dram_tensor("maskT", [SIN, 1], f32, kind="ExternalInput")
    wT = nc.dram_tensor("wT", [SIN, OC], f32, kind="ExternalInput")
    bias2d = nc.dram_tensor("bias2d", [128, 2], f32, kind="ExternalInput")
    w1d = nc.dram_tensor("w1d", [K, 9], f32, kind="ExternalInput")
    b1d = nc.dram_tensor("b1d", [K, 3], f32, kind="ExternalInput")
    w2d = nc.dram_tensor("w2d", [K, 9], f32, kind="ExternalInput")
    b2d = nc.dram_tensor("b2d", [K, 3], f32, kind="ExternalInput")
    w3bd = nc.dram_tensor("w3bd", [128, NT * OC], f32, kind="ExternalInput")
    outT = nc.dram_tensor("outT", [OC, B], f32, kind="ExternalOutput")

    modes = _exit_modes()

    with tile.TileContext(nc) as tc:
      for _rep in range(repeat):
        with (
            tc.tile_pool(name="const", bufs=1) as cp,
            tc.tile_pool(name="stage", bufs=3) as stp,
            tc.tile_pool(name="psumz", bufs=4, space="PSUM") as pp,
        ):
            # ---------- input loads ----------
            # HWDGE: small fp32 tensors
            v = cp.tile([K, B], f32)
            nc.sync.dma_start(v[:], xsatT.ap())
            w1s = cp.tile([K, 9], f32)
            nc.sync.dma_start(w1s[:], w1d.ap())
            b1s = cp.tile([K, 3], f32)
            nc.sync.dma_start(b1s[:], b1d.ap())
            w2s = cp.tile([K, 9], f32)
            nc.sync.dma_start(w2s[:], w2d.ap())
            b2s = cp.tile([K, 3], f32)
            nc.sync.dma_start(b2s[:], b2d.ap())
            bias2 = cp.tile([128, 2], f32)
            nc.sync.dma_start(bias2[:], bias2d.ap())
            masks = cp.tile([128, NCHUNK], f32)
            nc.sync.dma_start(masks[:], maskT.ap().rearrange("(c p) one -> p (c one)", p=128))

            # SWDGE cast loads (fp32 DRAM -> fp16 SBUF)
            x16a = cp.tile([128, 512 * NCHUNK], f16)
            w16a = cp.tile([128, OC * NCHUNK], f16)
            w3b = cp.tile([128, NT * OC], f16)
            QC = NCHUNK // 4

            def load_xw_quarter(q):
                nc.gpsimd.dma_start(
                    x16a[:, 512 * QC * q : 512 * QC * (q + 1)]
                        .rearrange("p (c b) -> p c b", b=B),
                    xT.ap()[128 * QC * q : 128 * QC * (q + 1), :]
                        .rearrange("(c p) b -> p c b", p=128))
                nc.gpsimd.dma_start(
                    w16a[:, OC * QC * q : OC * QC * (q + 1)]
                        .rearrange("p (c o) -> p c o", o=OC),
                    wT.ap()[128 * QC * q : 128 * QC * (q + 1), :]
                        .rearrange("(c p) o -> p c o", p=128))

            load_xw_quarter(0)
            load_xw_quarter(1)
            nc.gpsimd.dma_start(w3b[:], w3bd.ap())

            # prefetch the ACT relu table while DMAs run
            warm = cp.tile([1, 1], f32)
            nc.scalar.activation(warm[:], v[0:1, 0:1], AF.Relu)

            # c1[k, j] = (W1[k,0,j] + W1[k,1,j] + W1[k,2,j]) / 3
            c1a = cp.tile([K, 3], f32)
            nc.vector.tensor_tensor(c1a[:], w1s[:, 0:3], w1s[:, 3:6], AL.add)
            c1 = cp.tile([K, 3], f32)
            nc.vector.tensor_tensor(c1[:], c1a[:], w1s[:, 6:9], AL.add)
            c1s = cp.tile([K, 3], f32)
            nc.vector.tensor_scalar_mul(c1s[:], c1[:], 1.0 / 3.0)
            ones128 = cp.tile([128, 512], f16)
            nc.vector.memset(ones128[:], 1.0)

            # ---------- h path: v -> h1 -> h2 (layout [k, b]) ----------
            h1 = [cp.tile([K, B], f16, tag=f"h1_{j}", name=f"h1_{j}") for j in range(3)]
            for j in range(3):
                nc.scalar.activation(h1[j][:], v[:], AF.Relu,
                                     bias=b1s[:, j:j + 1], scale=c1s[:, j:j + 1])
            h2 = [cp.tile([K, B], f16, tag=f"h2_{j}", name=f"h2_{j}") for j in range(3)]
            for j in range(3):
                ma = stp.tile([K, B], f16, tag="hm0", name=f"hma{j}")
                nc.vector.tensor_scalar_mul(ma[:], h1[0][:], w2s[:, j : j + 1])
                mb = stp.tile([K, B], f16, tag="hm1", name=f"hmb{j}")
                nc.vector.tensor_scalar(mb[:], h1[1][:], w2s[:, 3 + j : 4 + j],
                                        b2s[:, j : j + 1], AL.mult, AL.add)
                sab = stp.tile([K, B], f16, tag="hm2", name=f"hsab{j}")
                nc.vector.tensor_tensor(sab[:], ma[:], mb[:], AL.add)
                mc = stp.tile([K, B], f16, tag="hm1", name=f"hmc{j}")
                nc.vector.tensor_scalar_mul(mc[:], h1[2][:], w2s[:, 6 + j : 7 + j])
                s = stp.tile([K, B], f16, tag="hm0", name=f"hs{j}")
                nc.vector.tensor_tensor(s[:], sab[:], mc[:], AL.add)
                nc.vector.tensor_scalar_max(h2[j][:], s[:], 0.0)

            # ---------- h2t pack: row 32g+j <- h2[j] (buds 32g+t), 4 DMAs ----
            h2t = cp.tile([128, 512 * NT], f16)       # [128, 16384]
            for j in range(3):
                nc.gpsimd.dma_start(
                    h2t[j::32, :].rearrange("p (t b) -> p t b", b=512),
                    h2[j][:],
                )
            nc.gpsimd.dma_start(
                h2t[3::32, :].rearrange("p (t b) -> p t b", b=512),
                ones128[:],
            )
            load_xw_quarter(2)
            load_xw_quarter(3)

            # ---------- dense: bursts through ring tiles, accumulated on DVE -
            live = [c for c in range(NCHUNK) if chunk_status[c] != "full"]
            dsb = cp.tile([128, 1024], f32)
            dgroups = [[c for c in live if 4 * q <= c < 4 * (q + 1)] for q in range(4)]
            dgroups = [g for g in dgroups if g]
            dense_first = [True]

            def emit_dense(gi):
                burst = dgroups[gi]
                dps = pp.tile([128, 1024], f32, tag="z", name=f"dense{gi}")
                for ci, c in enumerate(burst):
                    x16 = x16a[:, 512 * c : 512 * (c + 1)]
                    if chunk_status[c] == "partial":
                        xm = stp.tile([128, B], f16, tag="x16m", name=f"x16m_{c}_{_rep}")
                        nc.vector.tensor_scalar_mul(xm[:], x16, masks[:, c : c + 1])
                        x16 = xm[:]
                    for oc in range(2):
                        nc.tensor.matmul(dps[:, 512 * oc : 512 * (oc + 1)],
                                         w16a[:, OC * c + 128 * oc : OC * c + 128 * oc + 128],
                                         x16,
                                         start=ci == 0, stop=ci == len(burst) - 1)
                # accumulate burst into dsb (+bias once) in DVE's startup lull
                if dense_first[0]:
                    for oc in range(2):
                        nc.vector.tensor_scalar(
                            dsb[:, 512 * oc : 512 * (oc + 1)],
                            dps[:, 512 * oc : 512 * (oc + 1)],
                            bias2[:, oc : oc + 1], None, AL.add)
                    dense_first[0] = False
                else:
                    nc.vector.scalar_tensor_tensor(
                        dsb[:], dps[:], 0.0, dsb[:], op0=AL.add, op1=AL.add)

            # ---------- bud units ----------
            acc32 = cp.tile([128, 4096], f32)         # 2 oc x 2 lanes x 1024
            acc16 = cp.tile([128, 4096], f16)         # 2 oc x 2 lanes x 1024
            rbslab = cp.tile([128, 2 * RING * 1024], f16)   # 2 oc x RING slots
            n_act = [0, 0]
            first_d = [True, True, True, True]
            first_chain = [True, True]
            chained = [0, 0]
            odd_slots = {0: [], 1: []}

            act_total = [0, 0]
            for i, (t, oc_) in enumerate(_unit_order()):
                for half in range(2):
                    if modes[2 * i + half] == "A":
                        act_total[oc_] += 1

            def emit_chain(oc, base, nslots):
                # 2-slot cce-add sub-DMAs folding slab slots into acc16 lanes
                for s in range(0, nslots - 1, 2):
                    off = (oc * RING + base + s) * 1024
                    rb = rbslab[:, off : off + 2048]
                    aoc = acc16[:, 2048 * oc : 2048 * (oc + 1)]
                    if first_chain[oc]:
                        nc.gpsimd.dma_start(aoc, rb)
                        first_chain[oc] = False
                    else:
                        nc.gpsimd.dma_start(aoc, rb, accum_op=AL.add)
                if nslots % 2:
                    odd_slots[oc].append(base + nslots - 1)

            outsb = cp.tile([128, 1024], f32)

            def emit_tail(oc):
                # trailing slots fold via DVE TTs (no DMA latency), then lane
                # folds + final combine on DVE, then the output store.
                nleft = n_act[oc] - chained[oc]
                a16 = acc16[:, 2048 * oc : 2048 * (oc + 1)]
                for s in range(nleft):
                    slot = (chained[oc] + s) % RING
                    off = (oc * RING + slot) * 1024
                    nc.vector.tensor_tensor(a16[:, 0:1024], a16[:, 0:1024],
                                            rbslab[:, off : off + 1024], AL.add)
                for slot in odd_slots[oc]:
                    off = (oc * RING + slot) * 1024
                    nc.vector.tensor_tensor(a16[:, 0:1024], a16[:, 0:1024],
                                            rbslab[:, off : off + 1024], AL.add)
                nc.vector.tensor_tensor(a16[:, 0:1024], a16[:, 0:1024],
                                        a16[:, 1024:2048], AL.add)
                nc.vector.tensor_tensor(a16[:, 0:512], a16[:, 0:512],
                                        a16[:, 512:1024], AL.add)
                a32 = acc32[:, 2048 * oc : 2048 * (oc + 1)]
                nc.vector.tensor_tensor(a32[:, 0:1024], a32[:, 0:1024],
                                        a32[:, 1024:2048], AL.add)
                nc.vector.tensor_tensor(a32[:, 0:512], a32[:, 0:512],
                                        a32[:, 512:1024], AL.add)
                nc.vector.tensor_tensor(a32[:, 0:512], a32[:, 0:512],
                                        a16[:, 0:512], AL.add)
                ot = outsb[:, 512 * oc : 512 * (oc + 1)]
                nc.vector.tensor_tensor(ot, dsb[:, 512 * oc : 512 * (oc + 1)],
                                        a32[:, 0:512], AL.add)
                nc.sync.dma_start(outT.ap()[128 * oc : 128 * (oc + 1), :], ot)

            unit_order = _unit_order()
            last_unit = {0: max(i for i, (t, o) in enumerate(unit_order) if o == 0),
                         1: len(unit_order) - 1}

            emit_dense(0)
            if len(dgroups) > 1:
                emit_dense(1)
            dense_at = {2: 2, 6: 3}
            for i, (t, oc) in enumerate(unit_order):
                gi = dense_at.get(i)
                if gi is not None and gi < len(dgroups):
                    emit_dense(gi)
                for half in range(2):
                    e = 2 * i + half
                    zt = pp.tile([128, 1024], f32, tag="z", name=f"z{i}_{half}")
                    for gi in range(2):
                        g = 2 * half + gi
                        nc.tensor.matmul(
                            zt[:, 512 * gi : 512 * (gi + 1)],
                            w3b[32 * g : 32 * g + 4,
                                OC * t + 128 * oc : OC * t + 128 * oc + 128],
                            h2t[32 * g : 32 * g + 4, 512 * t : 512 * (t + 1)],
                            start=True, stop=True, tile_position=(32 * g, 0),
                        )
                    if modes[e] == "A":
                        slot = n_act[oc] % RING
                        dst = rbslab[:, (oc * RING + slot) * 1024 :
                                        (oc * RING + slot + 1) * 1024]
                        nc.scalar.activation(dst, zt[:], AF.Relu)
                        n_act[oc] += 1
                        if (n_act[oc] % GRP == 0
                                and n_act[oc] != act_total[oc]):
                            emit_chain(oc, (n_act[oc] - GRP) % RING, GRP)
                            chained[oc] = n_act[oc]
                    else:
                        lane = acc32[:, 2048 * oc + 1024 * half :
                                        2048 * oc + 1024 * (half + 1)]
                        if first_d[oc * 2 + half]:
                            nc.vector.tensor_scalar_max(lane, zt[:], 0.0)
                            first_d[oc * 2 + half] = False
                        else:
                            nc.vector.scalar_tensor_tensor(
                                lane, zt[:], 0.0, lane, op0=AL.max, op1=AL.add)
            emit_tail(0)
            emit_tail(1)
    nc.finalize()
    return nc


def _prep_inputs(x, sat_idx, weight, bias, W1, b1, W2, b2, W3, b3):
    """Host-side shard/layout prep (slicing/permutation only)."""
    x = np.ascontiguousarray(np.asarray(x, np.float32))
    sat = np.asarray(sat_idx).astype(np.int64)
    weight = np.asarray(weight, np.float32)
    bias = np.asarray(bias, np.float32)

    mask = np.ones(SIN, np.float32)
    mask[sat] = 0.0
    chunk_status = []
    for c in range(NCHUNK):
        mc = mask[128 * c : 128 * (c + 1)]
        if not mc.any():
            chunk_status.append("full")
        elif mc.all():
            chunk_status.append("clean")
        else:
            chunk_status.append("partial")
    chunk_status = tuple(chunk_status)

    xT = np.ascontiguousarray(x.T)                       # [SIN, B]
    xsatT = np.ascontiguousarray(x[:, sat].T)            # [K, B]
    maskT = np.ascontiguousarray(mask[:, None])          # [SIN, 1]
    w1h = np.ascontiguousarray(np.asarray(W1, np.float32).reshape(K, 9))
    w2h = np.ascontiguousarray(np.asarray(W2, np.float32).reshape(K, 9))
    b1h = np.ascontiguousarray(np.asarray(b1, np.float32))
    b2h = np.ascontiguousarray(np.asarray(b2, np.float32))
    W3 = np.asarray(W3, np.float32)
    b3 = np.asarray(b3, np.float32)

    in_maps = []
    for cidx in range(N_CORES):
        sl = slice(OC * cidx, OC * (cidx + 1))
        # w3b rows 32g+j hold W3[32g+t, j, sl] over supertiles t; row 32g+3 = b3
        w3bh = np.empty((128, NT, OC), np.float32)
        W3p = W3[:, :, sl]                                # [K, 3, OC]
        b3p = b3[:, sl]                                   # [K, OC]
        for g in range(4):
            buds = np.arange(NT) + 32 * g                 # k = 32g + t
            for j in range(3):
                w3bh[32 * g + j] = W3p[buds, j, :]
            w3bh[32 * g + 3] = b3p[buds, :]
        bias2 = np.ascontiguousarray(bias[sl].reshape(2, 128).T)  # [128, 2]
        in_maps.append({
            "xT": xT,
            "xsatT": xsatT,
            "maskT": maskT,
            "wT": np.ascontiguousarray(weight[sl, :].T),          # [SIN, OC]
            "bias2d": bias2,
            "w1d": w1h, "b1d": b1h, "w2d": w2h, "b2d": b2h,
            "w3bd": np.ascontiguousarray(w3bh.reshape(128, NT * OC)),
        })
    return chunk_status, in_maps


def kernel(**inputs) -> np.ndarray:
    from concourse.bass_utils import run_bass_kernel_spmd

    chunk_status, in_maps = _prep_inputs(
        inputs["x"], inputs["sat_idx"], inputs["weight"], inputs["bias"],
        inputs["W1"], inputs["b1"], inputs["W2"], inputs["b2"],
        inputs["W3"], inputs["b3"],
    )
    if chunk_status not in _compiled:
        _compiled[chunk_status] = _build(chunk_status)
    nc = _compiled[chunk_status]
    res = run_bass_kernel_spmd(nc, in_maps, core_ids=list(range(N_CORES)))
    outT = np.concatenate([res.results[c]["outT"] for c in range(N_CORES)], axis=0)
    return np.ascontiguousarray(outT.T).astype(np.float32)


# revision 60
# speedup vs baseline: 1.2908x; 1.0374x over previous
"""Trainium2 Bass kernel for nn_BuddingLayer (moe_routing).

Computation (B=512, SIN=SOUT=2048, K=128 buds):
  dense = (x * ~mask) @ weight.T + bias          mask = one-hot(sat_idx)
  per bud k (v = x[:, sat_idx[k]]):
    h1 = relu(v * c1[k] + b1[k])                 c1[k,j] = sum_i W1[k,i,j]/3
    h2 = relu(h1 @ W2[k] + b2[k])                [B, 3]
    u += relu(h2 @ W3[k] + b3[k])                [B, 2048]
  out = dense + u

Sharding: output-feature split, 256 columns per core (8 cores); compute in
transposed layout [o_part, b_free].  Host does slicing/permutation only; all
math runs on device.

v2 layout: supertile t packs buds {t, 32+t, 64+t, 96+t} at PE row-group bases
{0,32,64,96}; 4-row lhsT per bud carries W3 rows + b3 (constant-1 rhs row).
Dense runs first in its own PSUM scope; after it drains, bud z-units are
[128, 2048] (4 buds x 4 PSUM banks, double buffered).  Exits split between
ScalarE (relu -> fp16 slab slots, summed by SWDGE cce-add DMA trees + chained
DMA accumulate) and VectorE (fused relu+accumulate via scalar_tensor_tensor
into fp32 lanes).  Lane folds and final dense+bud combines run on GpSimd.
"""

import numpy as np

N_CORES = 8
B = 512
SIN = 2048
SOUT = 2048
K = 128
OC = SOUT // N_CORES          # 256 output cols per core
NCHUNK = SIN // 128           # 16 contraction chunks for dense
NT = K // 4                   # 32 supertiles (4 buds each)
NU = 2 * NT                   # 64 z-units: unit u -> oc=u%2, t=u//2

# tuning knobs
ACT_EXITS = 63                # body [128,1024] exits via ScalarE (rest: DVE)
TAILU = 4                     # per-oc trailing units grouped oc0-then-oc1
TAIL_A_UNITS = 2              # last units per oc -> Act relu + DVE TT folds
GRP = 4                       # Act slots (1024 wide) per cce-add chain DMA
RING = 16                     # slab-slot ring depth per oc

_compiled = {}


def _unit_order():
    return ([(t, oc) for t in range(NT - TAILU) for oc in (0, 1)]
            + [(t, 0) for t in range(NT - TAILU, NT)]
            + [(t, 1) for t in range(NT - TAILU, NT)])


def _exit_modes():
    """Per-exit 'A' (ScalarE relu -> slab slot) / 'D' (DVE fused relu+acc).
    Bresenham interleave keeps runs short so neither engine bursts; the last
    units of each oc go to DVE so no DMA chain dangles at the end."""
    order = _unit_order()
    ne = 2 * NU
    tail_units = set()
    for oc in (0, 1):
        idxs = [i for i, (t, o) in enumerate(order) if o == oc]
        tail_units.update(idxs[-TAIL_A_UNITS:])
    modes = ["D"] * ne
    body = [e for e in range(ne) if (e // 2) not in tail_units]
    n_act = min(ACT_EXITS, len(body))
    acc = 0
    for e in body:
        acc += n_act
        if acc >= len(body):
            acc -= len(body)
            modes[e] = "A"
    for u in tail_units:
        modes[2 * u] = modes[2 * u + 1] = "T"
    return modes


def _build(chunk_status, repeat=1):
    import concourse.bacc as bacc
    import concourse.mybir as mybir
    import concourse.tile as tile

    f32, f16 = mybir.dt.float32, mybir.dt.float16
    AL = mybir.AluOpType
    AF = mybir.ActivationFunctionType

    nc = bacc.Bacc("TRN2", target_bir_lowering=False, debug=False,
                   num_devices=N_CORES)

    # ---- DRAM I/O (per core) ----
    xT = nc.dram_tensor("xT", [SIN, B], f32, kind="ExternalInput")
    xsatT = nc.dram_tensor("xsatT", [K, B], f32, kind="ExternalInput")
    maskT = nc.dram_tensor("maskT", [SIN, 1], f32, kind="ExternalInput")
    wT = nc.dram_tensor("wT", [SIN, OC], f32, kind="ExternalInput")
    bias2d = nc.dram_tensor("bias2d", [128, 2], f32, kind="ExternalInput")
    w1d = nc.dram_tensor("w1d", [K, 9], f32, kind="ExternalInput")
    b1d = nc.dram_tensor("b1d", [K, 3], f32, kind="ExternalInput")
    w2d = nc.dram_tensor("w2d", [K, 9], f32, kind="ExternalInput")
    b2d = nc.dram_tensor("b2d", [K, 3], f32, kind="ExternalInput")
    w3bd = nc.dram_tensor("w3bd", [128, NT * OC], f32, kind="ExternalInput")
    outT = nc.dram_tensor("outT", [OC, B], f32, kind="ExternalOutput")

    modes = _exit_modes()

    with tile.TileContext(nc) as tc:
      for _rep in range(repeat):
        with (
            tc.tile_pool(name="const", bufs=1) as cp,
            tc.tile_pool(name="stage", bufs=3) as stp,
            tc.tile_pool(name="psumz", bufs=4, space="PSUM") as pp,
        ):
            # ---------- input loads ----------
            # HWDGE: small fp32 tensors
            w1s = cp.tile([K, 9], f32)
            nc.sync.dma_start(w1s[:], w1d.ap())
            v = cp.tile([K, B], f32)
            nc.sync.dma_start(v[:], xsatT.ap())
            b1s = cp.tile([K, 3], f32)
            nc.sync.dma_start(b1s[:], b1d.ap())
            w2s = cp.tile([K, 9], f32)
            nc.sync.dma_start(w2s[:], w2d.ap())
            b2s = cp.tile([K, 3], f32)
            nc.sync.dma_start(b2s[:], b2d.ap())
            bias2 = cp.tile([128, 2], f32)
            nc.sync.dma_start(bias2[:], bias2d.ap())
            masks = cp.tile([128, NCHUNK], f32)
            nc.sync.dma_start(masks[:], maskT.ap().rearrange("(c p) one -> p (c one)", p=128))

            # SWDGE cast loads (fp32 DRAM -> fp16 SBUF)
            x16a = cp.tile([128, 512 * NCHUNK], f16)
            w16a = cp.tile([128, OC * NCHUNK], f16)
            w3b = cp.tile([128, NT * OC], f16)
            QC = NCHUNK // 4

            def load_xw_quarter(q):
                nc.gpsimd.dma_start(
                    x16a[:, 512 * QC * q : 512 * QC * (q + 1)]
                        .rearrange("p (c b) -> p c b", b=B),
                    xT.ap()[128 * QC * q : 128 * QC * (q + 1), :]
                        .rearrange("(c p) b -> p c b", p=128))
                nc.gpsimd.dma_start(
                    w16a[:, OC * QC * q : OC * QC * (q + 1)]
                        .rearrange("p (c o) -> p c o", o=OC),
                    wT.ap()[128 * QC * q : 128 * QC * (q + 1), :]
                        .rearrange("(c p) o -> p c o", p=128))

            load_xw_quarter(0)
            load_xw_quarter(1)
            nc.gpsimd.dma_start(w3b[:], w3bd.ap())

            # prefetch the ACT relu table while DMAs run
            warm = cp.tile([1, 1], f32)
            nc.scalar.activation(warm[:], v[0:1, 0:1], AF.Relu)

            # c1[k, j] = (W1[k,0,j] + W1[k,1,j] + W1[k,2,j]) / 3
            c1a = cp.tile([K, 3], f32)
            nc.vector.tensor_tensor(c1a[:], w1s[:, 0:3], w1s[:, 3:6], AL.add)
            c1 = cp.tile([K, 3], f32)
            nc.vector.tensor_tensor(c1[:], c1a[:], w1s[:, 6:9], AL.add)
            c1s = cp.tile([K, 3], f32)
            nc.vector.tensor_scalar_mul(c1s[:], c1[:], 1.0 / 3.0)
            ones128 = cp.tile([128, 512], f16)
            nc.vector.memset(ones128[:], 1.0)

            # ---------- h path: v -> h1 -> h2 (layout [k, b]) ----------
            h1 = [cp.tile([K, B], f16, tag=f"h1_{j}", name=f"h1_{j}") for j in range(3)]
            for j in range(3):
                nc.scalar.activation(h1[j][:], v[:], AF.Relu,
                                     bias=b1s[:, j:j + 1], scale=c1s[:, j:j + 1])
            h2 = [cp.tile([K, B], f16, tag=f"h2_{j}", name=f"h2_{j}") for j in range(3)]
            for j in range(3):
                ma = stp.tile([K, B], f16, tag="hm0", name=f"hma{j}")
                nc.vector.tensor_scalar_mul(ma[:], h1[0][:], w2s[:, j : j + 1])
                mb = stp.tile([K, B], f16, tag="hm1", name=f"hmb{j}")
                nc.vector.tensor_scalar(mb[:], h1[1][:], w2s[:, 3 + j : 4 + j],
                                        b2s[:, j : j + 1], AL.mult, AL.add)
                sab = stp.tile([K, B], f16, tag="hm2", name=f"hsab{j}")
                nc.vector.tensor_tensor(sab[:], ma[:], mb[:], AL.add)
                mc = stp.tile([K, B], f16, tag="hm1", name=f"hmc{j}")
                nc.vector.tensor_scalar_mul(mc[:], h1[2][:], w2s[:, 6 + j : 7 + j])
                s = stp.tile([K, B], f16, tag="hm0", name=f"hs{j}")
                nc.vector.tensor_tensor(s[:], sab[:], mc[:], AL.add)
                nc.vector.tensor_scalar_max(h2[j][:], s[:], 0.0)

            # ---------- h2t pack: row 32g+j <- h2[j] (buds 32g+t), 4 DMAs ----
            h2t = cp.tile([128, 512 * NT], f16)       # [128, 16384]
            nc.gpsimd.dma_start(
                h2t[3::32, :].rearrange("p (t b) -> p t b", b=512),
                ones128[:],
            )
            for j in range(3):
                nc.gpsimd.dma_start(
                    h2t[j::32, :].rearrange("p (t b) -> p t b", b=512),
                    h2[j][:],
                )
            load_xw_quarter(2)
            load_xw_quarter(3)

            # ---------- dense: bursts through ring tiles, accumulated on DVE -
            live = [c for c in range(NCHUNK) if chunk_status[c] != "full"]
            dsb = cp.tile([128, 1024], f32)
            dgroups = [[c for c in live if 4 * q <= c < 4 * (q + 1)] for q in range(4)]
            dgroups = [g for g in dgroups if g]
            dense_first = [True]

            def emit_dense(gi):
                burst = dgroups[gi]
                dps = pp.tile([128, 1024], f32, tag="z", name=f"dense{gi}")
                for ci, c in enumerate(burst):
                    x16 = x16a[:, 512 * c : 512 * (c + 1)]
                    if chunk_status[c] == "partial":
                        xm = stp.tile([128, B], f16, tag="x16m", name=f"x16m_{c}_{_rep}")
                        nc.vector.tensor_scalar_mul(xm[:], x16, masks[:, c : c + 1])
                        x16 = xm[:]
                    for oc in range(2):
                        nc.tensor.matmul(dps[:, 512 * oc : 512 * (oc + 1)],
                                         w16a[:, OC * c + 128 * oc : OC * c + 128 * oc + 128],
                                         x16,
                                         start=ci == 0, stop=ci == len(burst) - 1)
                # accumulate burst into dsb (+bias once) in DVE's startup lull
                if dense_first[0]:
                    for oc in range(2):
                        nc.vector.tensor_scalar(
                            dsb[:, 512 * oc : 512 * (oc + 1)],
                            dps[:, 512 * oc : 512 * (oc + 1)],
                            bias2[:, oc : oc + 1], None, AL.add)
                    dense_first[0] = False
                else:
                    nc.vector.scalar_tensor_tensor(
                        dsb[:], dps[:], 0.0, dsb[:], op0=AL.add, op1=AL.add)

            # ---------- bud units ----------
            acc32 = cp.tile([128, 4096], f32)         # 2 oc x 2 lanes x 1024
            acc16 = cp.tile([128, 4096], f16)         # 2 oc x 2 lanes x 1024
            rbslab = cp.tile([128, 2 * RING * 1024], f16)   # 2 oc x RING slots
            n_act = [0, 0]
            first_d = [True, True, True, True]
            first_chain = [True, True]
            chained = [0, 0]
            odd_slots = {0: [], 1: []}

            act_total = [0, 0]
            for i, (t, oc_) in enumerate(_unit_order()):
                for half in range(2):
                    if modes[2 * i + half] == "A":
                        act_total[oc_] += 1
            tailslab = cp.tile([128, 8 * 1024], f16)
            n_tail = [0]

            def emit_chain(oc, base, nslots):
                # 2-slot cce-add sub-DMAs folding slab slots into acc16 lanes
                for s in range(0, nslots - 1, 2):
                    off = (oc * RING + base + s) * 1024
                    rb = rbslab[:, off : off + 2048]
                    aoc = acc16[:, 2048 * oc : 2048 * (oc + 1)]
                    if first_chain[oc]:
                        nc.gpsimd.dma_start(aoc, rb)
                        first_chain[oc] = False
                    else:
                        nc.gpsimd.dma_start(aoc, rb, accum_op=AL.add)
                if nslots % 2:
                    odd_slots[oc].append(base + nslots - 1)

            outsb = cp.tile([128, 1024], f32)

            def emit_tail(oc):
                # trailing slots fold via DVE TTs (no DMA latency), then lane
                # folds + final combine on DVE, then the output store.
                nleft = n_act[oc] - chained[oc]
                a16 = acc16[:, 2048 * oc : 2048 * (oc + 1)]
                for s in range(nleft):
                    slot = (chained[oc] + s) % RING
                    off = (oc * RING + slot) * 1024
                    nc.vector.tensor_tensor(a16[:, 0:1024], a16[:, 0:1024],
                                            rbslab[:, off : off + 1024], AL.add)
                for slot in odd_slots[oc]:
                    off = (oc * RING + slot) * 1024
                    nc.vector.tensor_tensor(a16[:, 0:1024], a16[:, 0:1024],
                                            rbslab[:, off : off + 1024], AL.add)
                nc.vector.tensor_tensor(a16[:, 0:1024], a16[:, 0:1024],
                                        a16[:, 1024:2048], AL.add)
                nc.vector.tensor_tensor(a16[:, 0:512], a16[:, 0:512],
                                        a16[:, 512:1024], AL.add)
                a32 = acc32[:, 2048 * oc : 2048 * (oc + 1)]
                nc.vector.tensor_tensor(a32[:, 0:1024], a32[:, 0:1024],
                                        a32[:, 1024:2048], AL.add)
                nc.vector.tensor_tensor(a32[:, 0:512], a32[:, 0:512],
                                        a32[:, 512:1024], AL.add)
                nc.vector.tensor_tensor(a32[:, 0:512], a32[:, 0:512],
                                        a16[:, 0:512], AL.add)
                ot = outsb[:, 512 * oc : 512 * (oc + 1)]
                nc.vector.tensor_tensor(ot, dsb[:, 512 * oc : 512 * (oc + 1)],
                                        a32[:, 0:512], AL.add)
                nc.sync.dma_start(outT.ap()[128 * oc : 128 * (oc + 1), :], ot)

            unit_order = _unit_order()
            last_unit = {0: max(i for i, (t, o) in enumerate(unit_order) if o == 0),
                         1: len(unit_order) - 1}

            emit_dense(0)
            if len(dgroups) > 1:
                emit_dense(1)
            dense_at = {4: 2, 8: 3}
            for i, (t, oc) in enumerate(unit_order):
                gi = dense_at.get(i)
                if gi is not None and gi < len(dgroups):
                    emit_dense(gi)
                for half in range(2):
                    e = 2 * i + half
                    zt = pp.tile([128, 1024], f32, tag="z", name=f"z{i}_{half}")
                    for gi in range(2):
                        g = 2 * half + gi
                        nc.tensor.matmul(
                            zt[:, 512 * gi : 512 * (gi + 1)],
                            w3b[32 * g : 32 * g + 4,
                                OC * t + 128 * oc : OC * t + 128 * oc + 128],
                            h2t[32 * g : 32 * g + 4, 512 * t : 512 * (t + 1)],
                            start=True, stop=True, tile_position=(32 * g, 0),
                        )
                    if modes[e] == "A":
                        slot = n_act[oc] % RING
                        dst = rbslab[:, (oc * RING + slot) * 1024 :
                                        (oc * RING + slot + 1) * 1024]
                        nc.scalar.activation(dst, zt[:], AF.Relu)
                        n_act[oc] += 1
                        if (n_act[oc] % GRP == 0
                                and n_act[oc] != act_total[oc]):
                            emit_chain(oc, (n_act[oc] - GRP) % RING, GRP)
                            chained[oc] = n_act[oc]
                    elif modes[e] == "T":
                        ts_off = n_tail[0] * 1024
                        n_tail[0] += 1
                        dst = tailslab[:, ts_off : ts_off + 1024]
                        nc.scalar.activation(dst, zt[:], AF.Relu)
                        a16 = acc16[:, 2048 * oc : 2048 * (oc + 1)]
                        nc.vector.tensor_tensor(a16[:, 0:1024], a16[:, 0:1024],
                                                dst, AL.add)
                    else:
                        lane = acc32[:, 2048 * oc + 1024 * half :
                                        2048 * oc + 1024 * (half + 1)]
                        if first_d[oc * 2 + half]:
                            nc.vector.tensor_scalar_max(lane, zt[:], 0.0)
                            first_d[oc * 2 + half] = False
                        else:
                            nc.vector.scalar_tensor_tensor(
                                lane, zt[:], 0.0, lane, op0=AL.max, op1=AL.add)
            emit_tail(0)
            emit_tail(1)
    nc.finalize()
    return nc


def _prep_inputs(x, sat_idx, weight, bias, W1, b1, W2, b2, W3, b3):
    """Host-side shard/layout prep (slicing/permutation only)."""
    x = np.ascontiguousarray(np.asarray(x, np.float32))
    sat = np.asarray(sat_idx).astype(np.int64)
    weight = np.asarray(weight, np.float32)
    bias = np.asarray(bias, np.float32)

    mask = np.ones(SIN, np.float32)
    mask[sat] = 0.0
    chunk_status = []
    for c in range(NCHUNK):
        mc = mask[128 * c : 128 * (c + 1)]
        if not mc.any():
            chunk_status.append("full")
        elif mc.all():
            chunk_status.append("clean")
        else:
            chunk_status.append("partial")
    chunk_status = tuple(chunk_status)

    xT = np.ascontiguousarray(x.T)                       # [SIN, B]
    xsatT = np.ascontiguousarray(x[:, sat].T)            # [K, B]
    maskT = np.ascontiguousarray(mask[:, None])          # [SIN, 1]
    w1h = np.ascontiguousarray(np.asarray(W1, np.float32).reshape(K, 9))
    w2h = np.ascontiguousarray(np.asarray(W2, np.float32).reshape(K, 9))
    b1h = np.ascontiguousarray(np.asarray(b1, np.float32))
    b2h = np.ascontiguousarray(np.asarray(b2, np.float32))
    W3 = np.asarray(W3, np.float32)
    b3 = np.asarray(b3, np.float32)

    in_maps = []
    for cidx in range(N_CORES):
        sl = slice(OC * cidx, OC * (cidx + 1))
        # w3b rows 32g+j hold W3[32g+t, j, sl] over supertiles t; row 32g+3 = b3
        w3bh = np.empty((128, NT, OC), np.float32)
        W3p = W3[:, :, sl]                                # [K, 3, OC]
        b3p = b3[:, sl]                                   # [K, OC]
        for g in range(4):
            buds = np.arange(NT) + 32 * g                 # k = 32g + t
            for j in range(3):
                w3bh[32 * g + j] = W3p[buds, j, :]
            w3bh[32 * g + 3] = b3p[buds, :]
        bias2 = np.ascontiguousarray(bias[sl].reshape(2, 128).T)  # [128, 2]
        in_maps.append({
            "xT": xT,
            "xsatT": xsatT,
            "maskT": maskT,
            "wT": np.ascontiguousarray(weight[sl, :].T),          # [SIN, OC]
            "bias2d": bias2,
            "w1d": w1h, "b1d": b1h, "w2d": w2h, "b2d": b2h,
            "w3bd": np.ascontiguousarray(w3bh.reshape(128, NT * OC)),
        })
    return chunk_status, in_maps


def kernel(**inputs) -> np.ndarray:
    from concourse.bass_utils import run_bass_kernel_spmd

    chunk_status, in_maps = _prep_inputs(
        inputs["x"], inputs["sat_idx"], inputs["weight"], inputs["bias"],
        inputs["W1"], inputs["b1"], inputs["W2"], inputs["b2"],
        inputs["W3"], inputs["b3"],
    )
    if chunk_status not in _compiled:
        _compiled[chunk_status] = _build(chunk_status)
    nc = _compiled[chunk_status]
    res = run_bass_kernel_spmd(nc, in_maps, core_ids=list(range(N_CORES)))
    outT = np.concatenate([res.results[c]["outT"] for c in range(N_CORES)], axis=0)
    return np.ascontiguousarray(outT.T).astype(np.float32)
